# revision 1
# baseline (speedup 1.0000x reference)
"""GCN (2-layer + pvt projection) Trainium2 kernel, 8-core SPMD.

Strategy: node (destination) parallelism across 8 cores. All index plumbing is
host-side: edges are bucketed per (dest-core, dest-tile, src-quarter), sorted by
dest, padded to 128-slot chunks. On device each SpMM is:
  dma_gather of source feature rows (256B rows, int16 indices into one of 4
  table quarters) -> banded value-matrix matmuls accumulating per dest tile in
  PSUM. Band matrices (val at (slot, dest-pos)) are host-built bf16 inputs.
Feature tables are bf16 [n, 128] (256B pitch) and are AllGather-ed between
layers. The SPMD program is made core-uniform by taking max/union of the
per-core chunk structure; per-core variation lives only in tensor contents.
"""

import sys

sys.path.insert(0, "/opt/trn_rl_repo")

import numpy as np
import ml_dtypes

from concourse import bass, bacc, mybir, tile
from concourse import bass_utils
from concourse.bass_utils import run_bass_kernel_spmd

# ---- NTFF profiling hook (normally injected by the launcher) -------------
# bass_utils reads antenv.axon_hooks.get_axon_ntff_profile_hook() when
# trace=True under axon; this image's antenv lacks that submodule, so build
# the ctypes equivalent here (mirrors trn_boot._ntff_profile_via_ctypes).


def _install_ntff_hook():
    import types
    import ctypes
    import contextlib

    if "antenv.axon_hooks" in sys.modules:
        return
    hook = None
    so_path = "/opt/axon/libaxon_pjrt.so"
    try:
        lib = ctypes.CDLL(so_path)
        if hasattr(lib, "axon_start_nrt_profile"):
            lib.axon_start_nrt_profile.argtypes = [
                ctypes.POINTER(ctypes.c_int64), ctypes.c_size_t]
            lib.axon_start_nrt_profile.restype = ctypes.c_int64
            lib.axon_stop_nrt_profile.argtypes = [ctypes.c_char_p]
            lib.axon_stop_nrt_profile.restype = ctypes.c_int64

            @contextlib.contextmanager
            def _hook(output_dir, device_ids):
                import jax
                jax.devices()
                if device_ids:
                    ids = (ctypes.c_int64 * len(device_ids))(*device_ids)
                    rc = lib.axon_start_nrt_profile(ids, len(device_ids))
                else:
                    rc = lib.axon_start_nrt_profile(None, 0)
                if rc != 0:
                    raise RuntimeError(f"axon_start_nrt_profile rc={rc}")
                try:
                    yield
                finally:
                    n = lib.axon_stop_nrt_profile(str(output_dir).encode())
                    print(f"ntff profile: {n} file(s) -> {output_dir}")

            hook = _hook
    except OSError:
        pass
    mod = types.ModuleType("antenv.axon_hooks")
    mod.get_axon_ntff_profile_hook = lambda: hook
    mod.set_axon_ntff_profile_hook = lambda h: None
    sys.modules["antenv.axon_hooks"] = mod


_install_ntff_hook()
# artifact upload needs monorepo S3 creds; keep artifacts local instead
bass_utils.upload_artifacts = lambda tmpdir: f"local://{tmpdir}"

BF16 = ml_dtypes.bfloat16
NCORES = 8
NQ = 4  # src-index quarters (int16 gather index limit)
P = 128

# full-problem dims
FULL = dict(N=100_000, NFEAT=512, NHID=64, NCLASS=40)


# --------------------------------------------------------------------------
# host-side planning
# --------------------------------------------------------------------------

class SpmmPlan:
    """Static (core-uniform) structure + per-core gather/band tensors."""

    def __init__(self, rows, cols, vals, n_loc, n_pad, G_T, full_span=False):
        n_tiles = n_loc // P
        qrows = n_pad // NQ
        assert qrows <= 32767
        self.n_tiles, self.qrows = n_tiles, qrows
        groups = [list(range(s, min(s + G_T, n_tiles)))
                  for s in range(0, n_tiles, G_T)]
        self.groups = groups

        # ---- shard by dest core, sort by (tile, quarter, dest) ----
        # feature tables are stored in plain global node order (row = node id)
        core = rows // n_loc
        per = []
        counts = np.zeros((NCORES, n_tiles, NQ), np.int64)
        trow = cols
        for k in range(NCORES):
            m = core == k
            d = (rows[m] - k * n_loc).astype(np.int64)
            c = trow[m].astype(np.int64)
            v = vals[m].astype(np.float32)
            t = d >> 7
            q = c // qrows
            o = np.lexsort((d, q, t))
            t, q, d, c, v = t[o], q[o], d[o], c[o], v[o]
            np.add.at(counts[k], (t, q), 1)
            per.append((t, q, d, c, v))

        chunks_tq = -(-counts.max(axis=0) // P)  # ceil
        # every tile needs >= 1 chunk (PSUM coverage via its first chunk)
        empty = chunks_tq.sum(axis=1) == 0
        chunks_tq[empty, 0] = 1
        slots_tq = chunks_tq * P

        # ---- slot layout: stream ordered by (group, quarter, tile) ----
        slot_base = np.zeros((n_tiles, NQ), np.int64)
        self.gq_off = {}      # (g,q) -> (slot_off, num_idxs)
        off = 0
        for g, tl in enumerate(groups):
            for q in range(NQ):
                b = off
                for t in tl:
                    slot_base[t, q] = off
                    off += slots_tq[t, q]
                self.gq_off[(g, q)] = (b, off - b)
        S = off
        self.S = S
        n_chunks = S // P

        # ---- per-core slot-aligned arrays ----
        src_loc = np.zeros((NCORES, S), np.int64)
        val_s = np.zeros((NCORES, S), np.float32)
        pos_s = np.zeros((NCORES, S), np.int64)
        valid = np.zeros((NCORES, S), bool)
        for k in range(NCORES):
            t, q, d, c, v = per[k]
            key = t * NQ + q
            # rank within (t,q) run (edges are sorted so runs are contiguous)
            ne = len(key)
            if ne:
                starts = np.r_[0, np.nonzero(np.diff(key))[0] + 1]
                run_id = np.zeros(ne, np.int64)
                run_id[starts[1:]] = 1
                run_id = np.cumsum(run_id)
                rank = np.arange(ne) - starts[run_id]
                s = slot_base[t, q] + rank
                src_loc[k, s] = c % qrows
                val_s[k, s] = v
                pos_s[k, s] = d & 127
                valid[k, s] = True

        # ---- union chunk spans ----
        lo = np.full(n_chunks, P, np.int64)
        hi = np.zeros(n_chunks, np.int64)
        cid = np.arange(S) // P
        for k in range(NCORES):
            m = valid[k]
            np.minimum.at(lo, cid[m], pos_s[k, m])
            np.maximum.at(hi, cid[m], pos_s[k, m] + 1)
        none = hi == 0
        lo[none] = 0
        hi[none] = 1
        if full_span:
            # row-major (band-as-lhsT) use: PSUM partition offsets are not
            # supported by the accumulation bookkeeping -> every chunk spans
            # the full 128-dest window
            lo[:] = 0
            hi[:] = P

        # ---- chunk metadata in processing order (g, t, q, c) ----
        # first chunk of each tile forced to full span [0,128) for PSUM
        # coverage; last chunk too so stop=True clears the whole bank group
        self.tile_chunks = {}  # t -> list of (q, colF, lhs_off, M, lo)
        chunk_ids = {}
        lhs_off = 0
        for g, tl in enumerate(groups):
            for t in tl:
                cis = []
                for q in range(NQ):
                    b0 = slot_base[t, q]
                    for c in range(chunks_tq[t, q]):
                        cis.append((q, b0, (b0 + c * P) // P))
                assert cis, f"tile {t} has no chunks"
                for ci in (cis[0][2], cis[-1][2]):
                    lo[ci], hi[ci] = 0, P
                lst = []
                for q, b0, ci in cis:
                    M = int(hi[ci] - lo[ci])
                    colF = (ci * P - self.gq_off[(g, q)][0]) // P
                    lst.append((q, int(colF), lhs_off, M, int(lo[ci])))
                    chunk_ids[ci] = lhs_off
                    lhs_off += M
                self.tile_chunks[t] = lst
        self.L = lhs_off

        # ---- per-core band (lhs) + index tensors ----
        lhs_off_arr = np.zeros(n_chunks, np.int64)
        for ci, o in chunk_ids.items():
            lhs_off_arr[ci] = o
        self.lhs_np = []
        self.idx_np = []
        for k in range(NCORES):
            m = valid[k]
            sl = np.nonzero(m)[0]
            lhs = np.zeros((P, self.L), np.float32)
            rowi = sl % P
            coli = lhs_off_arr[cid[sl]] + pos_s[k, sl] - lo[cid[sl]]
            assert (coli >= 0).all() and (coli < self.L).all()
            lhs[rowi, coli] = val_s[k, sl]
            self.lhs_np.append(lhs.astype(BF16))
            idx16 = np.zeros((16, S // 16), np.int16)
            ss = np.arange(S)
            idx16[ss % 16, ss // 16] = src_loc[k].astype(np.int16)
            self.idx_np.append(np.tile(idx16, (NCORES, 1)))  # [128, S//16]

        self.cmax = int(max(
            self.gq_off[(g, q)][1] // P
            for g in range(len(groups)) for q in range(NQ)))
        self.lg_max = int(max(
            sum(M for t in tl for (_, _, _, M, _) in self.tile_chunks[t])
            for tl in groups))
        self.imax = int(max(n // 16 for (_, n) in self.gq_off.values()))

    def group_lhs_span(self, g):
        tl = self.groups[g]
        o0 = self.tile_chunks[tl[0]][0][2]
        last = self.tile_chunks[tl[-1]][-1]
        return o0, last[2] + last[3] - o0


# --------------------------------------------------------------------------
# device kernel builder
# --------------------------------------------------------------------------

def build_kernel(dims, ep, pp):
    """ep/pp: SpmmPlan for adj and pvt. Returns compiled Bacc."""
    n_loc, n_pad = dims["n_loc"], dims["n_pad"]
    NFEAT, NHID, NCLASS = dims["NFEAT"], dims["NHID"], dims["NCLASS"]
    n_tiles = n_loc // P
    ncc = NFEAT // P
    qrows = n_pad // NQ
    f32 = mybir.dt.float32
    bf16 = mybir.dt.bfloat16
    i16 = mybir.dt.int16

    nc = bacc.Bacc("TRN2", target_bir_lowering=False, debug=False,
                   enable_asserts=False, num_devices=NCORES)

    x_d = nc.dram_tensor("x", [n_loc, NFEAT], bf16, kind="ExternalInput")
    w1_d = nc.dram_tensor("w1", [NFEAT, NHID], bf16, kind="ExternalInput")
    w2_d = nc.dram_tensor("w2", [NHID, NCLASS], bf16, kind="ExternalInput")
    b1_d = nc.dram_tensor("b1", [NHID, 1], f32, kind="ExternalInput")
    b2_d = nc.dram_tensor("b2c", [NCLASS, 1], f32, kind="ExternalInput")
    eidx_d = nc.dram_tensor("eidx", [P, ep.S // 16], i16, kind="ExternalInput")
    elhs_d = nc.dram_tensor("elhs", [P, ep.L], bf16, kind="ExternalInput")
    pidx_d = nc.dram_tensor("pidx", [P, pp.S // 16], i16, kind="ExternalInput")
    plhs_d = nc.dram_tensor("plhs", [P, pp.L], bf16, kind="ExternalInput")
    out_d = nc.dram_tensor("out", [n_loc, NCLASS], f32, kind="ExternalOutput")

    rg = [list(range(NCORES))]

    with tile.TileContext(nc) as tc:
        with (
            tc.tile_pool(name="dram", bufs=1, space="DRAM") as dram,
            tc.tile_pool(name="const", bufs=1) as cpool,
            tc.tile_pool(name="xt", bufs=4) as xtp,
            tc.tile_pool(name="fbuf", bufs=6) as fpool,
            tc.tile_pool(name="lhsb", bufs=2) as lpool,
            tc.tile_pool(name="idxb", bufs=4) as ipool,
            tc.tile_pool(name="stg", bufs=4) as spool,
            tc.tile_pool(name="psum", bufs=6, space="PSUM") as pspool,
        ):
            ag1_in = dram.tile([n_loc, P], bf16, tag="ag1_in")
            tab1 = dram.tile([n_pad, P], bf16, tag="tab1")
            ag2_in = dram.tile([n_loc, P], bf16, tag="ag2_in")
            tab2 = dram.tile([n_pad, P], bf16, tag="tab2")
            ag3_in = dram.tile([n_loc, P], bf16, tag="ag3_in")
            tab3 = dram.tile([n_pad, P], bf16, tag="tab3")

            # ---- constants ----
            w1_sb = cpool.tile([P, ncc, NHID], bf16, tag="w1")
            nc.sync.dma_start(
                out=w1_sb[:],
                in_=w1_d.ap().rearrange("(c p) f -> p c f", p=P))
            w2_sb = cpool.tile([NHID, NCLASS], bf16, tag="w2")
            nc.sync.dma_start(out=w2_sb[:], in_=w2_d.ap())
            b1_sb = cpool.tile([NHID, 1], f32, tag="b1")
            nc.sync.dma_start(out=b1_sb[:], in_=b1_d.ap())
            b2_sb = cpool.tile([NCLASS, 1], f32, tag="b2")
            nc.sync.dma_start(out=b2_sb[:], in_=b2_d.ap())
            ident = cpool.tile([P, P], bf16, tag="ident")
            from concourse.masks import make_identity
            make_identity(nc, ident[:])

            def emit_table_tile(ag_in, t, src_ap, nf):
                """stage psum/sbuf tile [*, nf] into the 256B-pitch table."""
                stg = spool.tile([P, P], bf16, tag="stg")
                nc.vector.memset(stg[:, nf:P], 0.0)
                nc.vector.tensor_copy(out=stg[:, 0:nf], in_=src_ap)
                nc.sync.dma_start(out=ag_in[t * P:(t + 1) * P, :], in_=stg[:])

            # ---- stage A: XW1 = x @ W1 (two node-half waves) ----
            half = n_tiles // 2
            for w, trange in enumerate((range(0, half),
                                        range(half, n_tiles))):
                nrows = (len(trange)) * P
                r0 = trange[0] * P
                xts = []
                for cc in range(ncc):
                    xt = xtp.tile([P, (n_tiles - half) * P], bf16, tag="xt")
                    nc.sync.dma_start_transpose(
                        out=xt[:, 0:nrows],
                        in_=x_d.ap()[r0:r0 + nrows, cc * P:(cc + 1) * P])
                    xts.append(xt)
                for t in trange:
                    ps = pspool.tile([P, NHID], f32, tag="ps")
                    for cc in range(ncc):
                        nc.tensor.matmul(
                            out=ps[:],
                            lhsT=xts[cc][:, t * P - r0:(t + 1) * P - r0],
                            rhs=w1_sb[:, cc, :],
                            start=(cc == 0), stop=(cc == ncc - 1))
                    emit_table_tile(ag1_in, t, ps[:], NHID)
            nc.gpsimd.collective_compute(
                "AllGather", mybir.AluOpType.bypass, replica_groups=rg,
                ins=[ag1_in[:].opt()], outs=[tab1[:].opt()])

            # ---- generic SpMM pass ----
            def spmm(plan, tab, idx_d, lhs_d, mode, nf, consume):
                """mode "T": feature-major psum [nf, 128] (band as rhs, free
                offsets). mode "A": row-major psum [128, nf] (band as lhsT,
                full-span chunks only). consume(t, psum_ap) moves psum out."""
                for g, tl in enumerate(plan.groups):
                    fbs = {}
                    for q in range(NQ):
                        soff, n_idx = plan.gq_off[(g, q)]
                        isb = ipool.tile([P, plan.imax], i16, tag="idx")
                        nc.sync.dma_start(
                            out=isb[:, 0:n_idx // 16],
                            in_=idx_d.ap()[:, soff // 16:(soff + n_idx) // 16])
                        fb = fpool.tile([P, plan.cmax, P], bf16, tag="F")
                        nc.gpsimd.dma_gather(
                            fb[:, 0:n_idx // P, :],
                            tab[q * qrows:(q + 1) * qrows, :],
                            isb[:, 0:n_idx // 16],
                            n_idx, n_idx, P, elem_step=P,
                            single_packet=False)
                        fbs[q] = fb
                    lo0, lg = plan.group_lhs_span(g)
                    lsb = lpool.tile([P, plan.lg_max], bf16, tag="lhs")
                    nc.sync.dma_start(out=lsb[:, 0:lg],
                                      in_=lhs_d.ap()[:, lo0:lo0 + lg])
                    for t in tl:
                        chunks = plan.tile_chunks[t]
                        prev = None
                        if mode == "T":  # feature-major out [nf, 128]
                            ps = pspool.tile([nf, P], f32, tag="ps")
                            for i, (q, colF, loff, M, lo) in enumerate(chunks):
                                mm = nc.tensor.matmul(
                                    out=ps[:, lo:lo + M],
                                    lhsT=fbs[q][:, colF, 0:nf],
                                    rhs=lsb[:, loff - lo0:loff - lo0 + M],
                                    start=(i == 0),
                                    stop=(i == len(chunks) - 1))
                                if prev is not None:
                                    tile.add_dep_helper(mm.ins, prev.ins,
                                                        sync=False,
                                                        reason="acc order")
                                prev = mm
                        else:  # row-major out [128, nf]; full-span chunks
                            ps = pspool.tile([P, nf], f32, tag="ps")
                            for i, (q, colF, loff, M, lo) in enumerate(chunks):
                                assert lo == 0 and M == P
                                mm = nc.tensor.matmul(
                                    out=ps[:],
                                    lhsT=lsb[:, loff - lo0:loff - lo0 + M],
                                    rhs=fbs[q][:, colF, 0:nf],
                                    start=(i == 0),
                                    stop=(i == len(chunks) - 1))
                                if prev is not None:
                                    tile.add_dep_helper(mm.ins, prev.ins,
                                                        sync=False,
                                                        reason="acc order")
                                prev = mm
                        consume(t, ps)

            # ---- gc1: h1^T = relu(spmm(adj, XW1) + b1), feature-major ----
            h1T = cpool.tile([NHID, n_loc], bf16, tag="h1T")

            def gc1_consume(t, ps):
                nc.scalar.activation(
                    out=h1T[:, t * P:(t + 1) * P], in_=ps[:],
                    func=mybir.ActivationFunctionType.Relu,
                    bias=b1_sb[:, 0:1], scale=1.0)

            spmm(ep, tab1, eidx_d, elhs_d, "T", NHID, gc1_consume)

            # ---- B2: H2pre = h1 @ W2 (row-major) ----
            for t in range(n_tiles):
                ps = pspool.tile([P, NCLASS], f32, tag="ps")
                nc.tensor.matmul(out=ps[:], lhsT=h1T[:, t * P:(t + 1) * P],
                                 rhs=w2_sb[:], start=True, stop=True)
                emit_table_tile(ag2_in, t, ps[:], NCLASS)
            nc.gpsimd.collective_compute(
                "AllGather", mybir.AluOpType.bypass, replica_groups=rg,
                ins=[ag2_in[:].opt()], outs=[tab2[:].opt()])

            # ---- gc2: h2 = spmm(adj, H2pre) + b2, then per-tile transpose
            def gc2_consume(t, ps):
                h2t = spool.tile([NHID, P], bf16, tag="h2t")
                nc.vector.memset(h2t[32:NHID, :], 0.0)
                nc.vector.tensor_scalar(
                    out=h2t[0:NCLASS, :], in0=ps[:],
                    scalar1=b2_sb[:, 0:1], scalar2=None,
                    op0=mybir.AluOpType.add)
                pst = pspool.tile([P, NHID], bf16, tag="ps")
                nc.tensor.transpose(out=pst[:], in_=h2t[:],
                                    identity=ident[0:NHID, 0:NHID])
                emit_table_tile(ag3_in, t, pst[:], NHID)

            spmm(ep, tab2, eidx_d, elhs_d, "T", NCLASS, gc2_consume)
            nc.gpsimd.collective_compute(
                "AllGather", mybir.AluOpType.bypass, replica_groups=rg,
                ins=[ag3_in[:].opt()], outs=[tab3[:].opt()])

            # ---- pvt spmm + per-tile log_softmax ----
            def pvt_consume(t, ps):
                mxt = spool.tile([P, 1], f32, tag="mxt")
                nc.vector.tensor_reduce(out=mxt[:], in_=ps[:],
                                        axis=mybir.AxisListType.X,
                                        op=mybir.AluOpType.max)
                sh = spool.tile([P, NCLASS], f32, tag="sh")
                nc.vector.tensor_scalar(
                    out=sh[:], in0=ps[:], scalar1=mxt[:, 0:1], scalar2=None,
                    op0=mybir.AluOpType.subtract)
                eb = spool.tile([P, NCLASS], f32, tag="eb")
                st = spool.tile([P, 1], f32, tag="st")
                nc.scalar.activation(out=eb[:], in_=sh[:],
                                     func=mybir.ActivationFunctionType.Exp,
                                     accum_out=st[:, 0:1])
                lst = spool.tile([P, 1], f32, tag="lst")
                nc.scalar.activation(out=lst[:], in_=st[:],
                                     func=mybir.ActivationFunctionType.Ln)
                ob = spool.tile([P, NCLASS], f32, tag="ob")
                nc.vector.tensor_scalar(
                    out=ob[:], in0=sh[:], scalar1=lst[:, 0:1], scalar2=None,
                    op0=mybir.AluOpType.subtract)
                nc.sync.dma_start(out=out_d.ap()[t * P:(t + 1) * P, :],
                                  in_=ob[:])

            spmm(pp, tab3, pidx_d, plhs_d, "A", NCLASS, pvt_consume)

    nc.compile()
    return nc


def _run(inputs, dims, G_T=10, trace=True):
    N = dims["N"]
    NFEAT, NHID, NCLASS = dims["NFEAT"], dims["NHID"], dims["NCLASS"]
    n_loc = -(-N // (NCORES * P)) * P
    n_pad = n_loc * NCORES
    dims = dict(dims, n_loc=n_loc, n_pad=n_pad)

    ep = SpmmPlan(inputs["adj_row"].astype(np.int64),
                  inputs["adj_col"].astype(np.int64),
                  np.asarray(inputs["adj_val"], np.float32),
                  n_loc, n_pad, G_T)
    pvt_gt = max(G_T, 2 * G_T)
    pp = SpmmPlan(inputs["pvt_row"].astype(np.int64),
                  inputs["pvt_col"].astype(np.int64),
                  np.asarray(inputs["pvt_val"], np.float32),
                  n_loc, n_pad, G_T, full_span=True)

    nc = build_kernel(dims, ep, pp)

    x_pad = np.zeros((n_pad, NFEAT), BF16)
    x_pad[:N] = np.asarray(inputs["x"], np.float32).astype(BF16)
    w1 = np.asarray(inputs["W1"], np.float32).astype(BF16)
    w2 = np.asarray(inputs["W2"], np.float32).astype(BF16)
    b1 = np.asarray(inputs["b1"], np.float32).reshape(NHID, 1)
    b2c = np.asarray(inputs["b2"], np.float32).reshape(NCLASS, 1).copy()

    in_maps = []
    for k in range(NCORES):
        in_maps.append({
            "x": x_pad[k * n_loc:(k + 1) * n_loc],
            "w1": w1, "w2": w2, "b1": b1, "b2c": b2c,
            "eidx": ep.idx_np[k], "elhs": ep.lhs_np[k],
            "pidx": pp.idx_np[k], "plhs": pp.lhs_np[k],
        })

    res = run_bass_kernel_spmd(nc, in_maps, core_ids=list(range(NCORES)),
                               trace=trace)
    _run.last_exec_time_ns = res.exec_time_ns
    out = np.concatenate([r["out"] for r in res.results], axis=0)[:N]
    return np.ascontiguousarray(out.astype(np.float32))


_run.last_exec_time_ns = None


def kernel(**inputs) -> np.ndarray:
    return _run(inputs, FULL)

